# revision 6
# baseline (speedup 1.0000x reference)
"""Trainium2 Bass kernel for AdaptiveCantorModalityFusion.

Strategy: data-parallel over batch across 8 NeuronCores (2 batches/core,
weights replicated, no collectives). On-chip pipeline per core:

  x (host-pretransposed, feature-major) -> p = x@Wp + bp -> gate MLP ->
  z = p * (a*gate + 1-a)  -> qkv = z@Wqkv + (emb@Wqkv + bqkv)  ->
  pairwise 2-way softmax attention (clip_l<->t5_l, clip_g<->t5_g) ->
  out = ctx@Wout  (token-major, direct DMA out)

The reference's 4-source masked softmax collapses to a 2-way softmax:
w_self = sigmoid((d_self - d_cross)/c - beta_pair). Padded positions of
the short (clip) modalities contribute K=bk=0 / V=bv=0, so for t5 target
positions s>=77 the cross score is 0 and the partner V vanishes.

v2 changes vs the 284us baseline:
- gate MLP matmuls in fp8e4 DoubleRow (weights host-scaled x64); gelu
  replaced by x*sigmoid(1.702x) so the whole kernel uses only the
  Sigmoid ACT table (kills 8x 1.28us ACT_TABLE_LOAD thrash).
- clip_l+clip_g share one z tile -> single 308-col qkv matmuls.
- modality order [0,2,3,1]: tiny clip_l first (fast PE start), each
  gate chain hides under the next modality's projection matmuls.
- qkv order t5_l -> clip(+pair0 products) -> t5_g(+pair1 products) with
  score/rep/ctx stages threaded between so PE never waits on DVE/ACT,
  and wout stages spread so output DMA isn't a tail burst.
"""

import numpy as np
import ml_dtypes

B, S, D, H, HD, M = 16, 256, 1024, 16, 64, 4
DIMS = [768, 1280, 2048, 2048]
SEQS = [77, 77, 256, 256]
NCORES = 8
BL = B // NCORES                    # 2 batches per core
TOKS = [BL * s for s in SEQS]       # [154, 154, 512, 512]
KCH = [d // 128 for d in DIMS]      # [6, 10, 16, 16]
OUT_OFF = [0, 77, 154, 410]
TOTSEQ = sum(SEQS)                  # 666
NQC = 3 * D // 128                  # 24 qkv output chunks
PAIRS = [(0, 2), (1, 3)]
S_G = 64.0                          # host scale on gate weights (fp8)
TCLIP = TOKS[0] + TOKS[1]           # 308 merged clip tokens

BF16 = ml_dtypes.bfloat16
F8 = ml_dtypes.float8_e4m3

_cache = {}


def _build(cinv, nbeta, a_gate):
    """Build the per-core Bass program. cinv/nbeta/a_gate are python floats
    baked into the instruction stream (they come from scalar inputs)."""
    import sys
    if '/opt/trn_rl_repo' not in sys.path:
        sys.path.insert(0, '/opt/trn_rl_repo')
    import concourse.bass as bass
    import concourse.mybir as mybir
    from concourse import bacc
    from concourse.tile import TileContext

    dt = mybir.dt
    AF = mybir.ActivationFunctionType
    DR = mybir.MatmulPerfMode.DoubleRow

    nc = bacc.Bacc("TRN2", target_bir_lowering=False, debug=False,
                   num_devices=NCORES)

    # ---- DRAM parameters ----
    xp = [nc.declare_dram_parameter(f"x{m}", [DIMS[m], TOKS[m]], dt.bfloat16,
                                    isOutput=False) for m in range(M)]
    wp = [nc.declare_dram_parameter(f"wp{m}", [DIMS[m], D], dt.bfloat16,
                                    isOutput=False) for m in range(M)]
    wg18 = nc.declare_dram_parameter("wg18", [128, M * 2048], dt.float8e4, isOutput=False)
    wg28 = nc.declare_dram_parameter("wg28", [128, M * 256], dt.float8e4, isOutput=False)
    wqkv = nc.declare_dram_parameter("wqkv", [D, 3 * D], dt.bfloat16, isOutput=False)
    wout = nc.declare_dram_parameter("wout", [M * D, D], dt.bfloat16, isOutput=False)
    constf = nc.declare_dram_parameter("constf", [128, 142], dt.float32, isOutput=False)
    constb = nc.declare_dram_parameter("constb", [128, 1152], dt.bfloat16, isOutput=False)
    out = nc.declare_dram_parameter("out", [BL * TOTSEQ, D], dt.bfloat16, isOutput=True)

    with TileContext(nc) as tc:
        with tc.tile_pool(name="const", bufs=1) as constp, \
             tc.tile_pool(name="psum", bufs=8, space="PSUM") as psump, \
             tc.tile_pool(name="qkv", bufs=1) as qkvp:
            pzp_cm = tc.tile_pool(name="pz", bufs=1, side="right")
            pzp = pzp_cm.__enter__()
            wqkvp_cm = tc.tile_pool(name="wqkvp", bufs=1, side="right")
            wqkvp = wqkvp_cm.__enter__()
            p8p_cm = tc.tile_pool(name="p8", bufs=1, side="right")
            p8p = p8p_cm.__enter__()

            cf_t = constp.tile([128, 142], dt.float32, tag="cf")
            cb_t = constp.tile([128, 1152], dt.bfloat16, tag="cb")
            bp_t = cf_t[:, 0:32].rearrange("p (m c) -> p m c", m=M)
            bg1s_t = cf_t[:, 32:40].rearrange("p (m c) -> p m c", m=M)  # 1.702*bg1
            bg2_t = cf_t[:, 40:44].rearrange("p (m c) -> p m c", m=M)
            bqkv_t = cf_t[:, 44:140].rearrange("p (m c) -> p m c", m=M)
            nb_t = cf_t[:, 140:142]
            seg_t = cb_t[:, 0:128].rearrange("p (k c) -> p k c", k=8)
            segt_t = cb_t[:, 128:1152].rearrange("p (k c) -> p k c", k=8)

            wq_t = wqkvp.tile([128, 8, 3 * D], dt.bfloat16, tag="wqkv")
            wqin = wqkv.ap().rearrange("(k p) n -> p k n", p=128)

            # z tiles: clips merged [.., 308]; per-modality slices
            pz01 = pzp.tile([128, 8, TCLIP], dt.bfloat16, tag="pz01", name="pz01")
            pz2 = pzp.tile([128, 8, TOKS[2]], dt.bfloat16, tag="pz2", name="pz2")
            pz3 = pzp.tile([128, 8, TOKS[3]], dt.bfloat16, tag="pz3", name="pz3")
            p8_01 = p8p.tile([128, 8, TCLIP], dt.float8e4, tag="p801", name="p801")
            p8_2 = p8p.tile([128, 8, TOKS[2]], dt.float8e4, tag="p82", name="p82")
            p8_3 = p8p.tile([128, 8, TOKS[3]], dt.float8e4, tag="p83", name="p83")

            def zsl(m):          # (bf16 z view, fp8 p view) for modality m
                if m == 0:
                    return pz01[:, :, 0:TOKS[0]], p8_01[:, :, 0:TOKS[0]]
                if m == 1:
                    return pz01[:, :, TOKS[0]:TCLIP], p8_01[:, :, TOKS[0]:TCLIP]
                return (pz2, p8_2) if m == 2 else (pz3, p8_3)

            qk = {}

            # ---- stages A-C: load x.T, project, gate ----
            g8p_cm = tc.tile_pool(name="g8p", bufs=1)
            g8p = g8p_cm.__enter__()
            wg18_t = g8p.tile([128, M * 2048], dt.float8e4, tag="wg18")
            wg28_t = g8p.tile([128, M * 256], dt.float8e4, tag="wg28")
            wg18_v = wg18_t[:].rearrange("p (m j h two c) -> p m j h two c",
                                         m=M, j=4, h=2, two=2)
            wg28_v = wg28_t[:].rearrange("p (m two c) -> p m two c", m=M, two=2)

            def gate_stage(m, gtp):
                """h = (p@Wg1)*sig(1.702*(p@Wg1)); gate=sig(h@Wg2); z=p*(a*g+1-a)"""
                T = TOKS[m]
                pz_m, p8_m = zsl(m)
                h_ps = [psump.tile([128, 512], dt.float32, tag="bank",
                                   name="hpsum")[:, :T] for _ in range(2)]
                for hc in range(2):
                    for j in range(4):
                        nc.tensor.matmul(h_ps[hc], wg18_v[:, m, j, hc],
                                         p8_m[:, 2 * j:2 * j + 2, :],
                                         start=(j == 0), stop=(j == 3),
                                         perf_mode=DR)
                sg1 = gtp.tile([128, 2, 512], dt.bfloat16, tag="sg1",
                               name="sg1")[:, :, :T]
                h8 = gtp.tile([128, 2, 512], dt.float8e4, tag="h8",
                              name="h8")[:, :, :T]
                for hc in range(2):
                    nc.scalar.activation(sg1[:, hc, :], h_ps[hc], AF.Sigmoid,
                                         bias=bg1s_t[:, m, hc:hc + 1],
                                         scale=float(1.702 / S_G))
                    nc.vector.scalar_tensor_tensor(
                        h8[:, hc, :], h_ps[hc], float(1.0 / S_G), sg1[:, hc, :],
                        mybir.AluOpType.mult, mybir.AluOpType.mult)
                g_ps = psump.tile([128, 512], dt.float32, tag="bank",
                                  name="gpsum")[:, :T]
                nc.tensor.matmul(g_ps, wg28_v[:, m], h8[:], start=True,
                                 stop=True, perf_mode=DR)
                sg = gtp.tile([128, 512], dt.float32, tag="sg", name="sg")[:, :T]
                nc.scalar.activation(sg, g_ps, AF.Sigmoid,
                                     bias=bg2_t[:, m, 0:1], scale=float(1.0 / S_G))
                sc = gtp.tile([128, 512], dt.bfloat16, tag="sc", name="sc")[:, :T]
                nc.vector.tensor_scalar(sc, sg, float(a_gate[m]),
                                        float(1.0 - a_gate[m]),
                                        mybir.AluOpType.mult, mybir.AluOpType.add)
                for mc in range(8):
                    nc.vector.tensor_mul(pz_m[:, mc, :], pz_m[:, mc, :], sc)

            with tc.tile_pool(name="xts", bufs=2) as xtsp, \
                 tc.tile_pool(name="xtb", bufs=1) as xtbp, \
                 tc.tile_pool(name="wpp", bufs=6) as wpp, \
                 tc.tile_pool(name="gt", bufs=2) as gtp:
                WQ_AFTER = {1: (0, 4), 2: (4, 8)}   # wq chunk loads after mi
                for mi, m in enumerate([0, 2, 3, 1]):
                    T, KC = TOKS[m], KCH[m]
                    pz_m, p8_m = zsl(m)
                    xt_m = (xtsp if m <= 1 else xtbp).tile(
                        [128, KC, T], dt.bfloat16, tag="xt")
                    xin = xp[m].ap().rearrange("(k p) t -> p k t", p=128)
                    wpin = wp[m].ap().rearrange("(k p) n -> p k n", p=128)
                    p_ps = [psump.tile([128, 512], dt.float32, tag="bank",
                                       name="ppsum")[:, :T] for _ in range(8)]
                    for kc in range(KC):
                        nc.sync.dma_start(out=xt_m[:, kc, :], in_=xin[:, kc, :])
                        wp_k = wpp.tile([128, D], dt.bfloat16, tag="wpc", name="wpk")
                        nc.sync.dma_start(wp_k[:], wpin[:, kc, :])
                        if mi == 0 and kc == 0:
                            # constants right after first x/wp issue (tiny)
                            nc.sync.dma_start(cf_t[:], constf.ap())
                        if mi == 0 and kc == KC - 1:
                            # gate weights after all of m0's x/wp issues
                            nc.sync.dma_start(wg18_t[:], wg18.ap())
                            nc.sync.dma_start(wg28_t[:], wg28.ap())
                        if mi == 2 and kc == 0:
                            nc.sync.dma_start(cb_t[:], constb.ap())
                        for mc in range(8):
                            nc.tensor.matmul(p_ps[mc], wp_k[:, mc * 128:(mc + 1) * 128],
                                             xt_m[:, kc, :],
                                             start=(kc == 0), stop=(kc == KC - 1))
                    for mc in range(8):
                        # two independent evictions of the same PSUM:
                        # bf16 z-precursor on ACT, fp8 gate input on DVE
                        nc.scalar.add(pz_m[:, mc, :], p_ps[mc], bp_t[:, m, mc:mc + 1])
                        nc.vector.tensor_scalar_add(p8_m[:, mc, :], p_ps[mc],
                                                    bp_t[:, m, mc:mc + 1])
                    gate_stage(m, gtp)
                    if mi in WQ_AFTER:
                        lo, hi = WQ_AFTER[mi]
                        for wkc in range(lo, hi):
                            nc.sync.dma_start(wq_t[:, wkc, :], wqin[:, wkc, :])
            g8p_cm.__exit__(None, None, None)
            p8p_cm.__exit__(None, None, None)

            wo2p_cm = tc.tile_pool(name="wo2p", bufs=1)
            wo2p = wo2p_cm.__enter__()
            wo2_t = wo2p.tile([128, 8, D], dt.bfloat16, tag="wo2", name="wo2")
            nc.sync.dma_start(wo2_t[:], wout.ap()[2 * D:3 * D, :]
                              .rearrange("(k p) n -> p k n", p=128))

            # ---- stages D-F ----
            def aview(ap3):
                return ap3.rearrange("p (b s) -> p b s", b=BL)

            def bviewv(ap3, SA):
                return ap3.rearrange("p (b s) -> p b s", b=BL)[:, :, :SA]

            def qsl(m, j):       # qkv chunk j of modality m
                if m == 0:
                    return qk[0][:, j, 0:TOKS[0]]
                if m == 1:
                    return qk[0][:, j, TOKS[0]:TCLIP]
                return qk[m][:, j, :]

            prodp_cm = tc.tile_pool(name="prods", bufs=1)
            prodp = prodp_cm.__enter__()
            repp_cm = tc.tile_pool(name="reps", bufs=1)
            repp = repp_cm.__enter__()

            def qkv_t5(m, oc_range, prods_pi=None, prods=None):
                z_m, _ = zsl(m)
                for oc in oc_range:
                    q_ps = psump.tile([128, 512], dt.float32, tag="bank",
                                      name="qpsum")
                    for kc in range(8):
                        nc.tensor.matmul(q_ps, wq_t[:, kc, oc * 128:(oc + 1) * 128],
                                         z_m[:, kc, :],
                                         start=(kc == 0), stop=(kc == 7))
                    nc.scalar.add(qk[m][:, oc, :], q_ps, bqkv_t[:, m, oc:oc + 1])
                    if prods_pi is not None and 8 <= oc < 16:
                        make_prods(prods_pi, oc - 8, prods)

            def qkv_clip(prods_pi=None, prods=None):
                for oc in range(NQC):
                    q_ps = psump.tile([128, 512], dt.float32, tag="bank",
                                      name="qpsum")[:, :TCLIP]
                    for kc in range(8):
                        nc.tensor.matmul(q_ps, wq_t[:, kc, oc * 128:(oc + 1) * 128],
                                         pz01[:, kc, :],
                                         start=(kc == 0), stop=(kc == 7))
                    nc.scalar.add(qk[0][:, oc, 0:TOKS[0]], q_ps[:, 0:TOKS[0]],
                                  bqkv_t[:, 0, oc:oc + 1])
                    nc.scalar.add(qk[0][:, oc, TOKS[0]:TCLIP], q_ps[:, TOKS[0]:TCLIP],
                                  bqkv_t[:, 1, oc:oc + 1])
                    if prods_pi is not None and 8 <= oc < 16:
                        make_prods(prods_pi, oc - 8, prods)

            def make_prods(pi, kc, prods):
                A, Bm = PAIRS[pi]
                SA = SEQS[A]
                pAA = prodp.tile([128, 154], dt.bfloat16, tag=f"paa{kc}", name="paa")
                nc.vector.tensor_mul(pAA, qsl(A, kc), qsl(A, 8 + kc))
                pAB = prodp.tile([128, 154], dt.bfloat16, tag=f"pab{kc}", name="pab")
                nc.vector.scalar_tensor_tensor(
                    aview(pAB), aview(qsl(A, kc)), -1.0,
                    bviewv(qsl(Bm, 8 + kc), SA),
                    mybir.AluOpType.mult, mybir.AluOpType.mult)
                pBB = prodp.tile([128, 512], dt.bfloat16, tag=f"pbb{kc}", name="pbb")
                nc.vector.tensor_mul(pBB, qsl(Bm, kc), qsl(Bm, 8 + kc))
                pBA = prodp.tile([128, 154], dt.bfloat16, tag=f"pba{kc}", name="pba")
                nc.vector.scalar_tensor_tensor(
                    aview(pBA), bviewv(qsl(Bm, kc), SA), -1.0,
                    aview(qsl(A, 8 + kc)),
                    mybir.AluOpType.mult, mybir.AluOpType.mult)
                prods[kc] = (pAA, pAB, pBB, pBA)

            def score_sig(pi, prods, wA_t, wB_t):
                A, Bm = PAIRS[pi]
                TA, TB, SA = TOKS[A], TOKS[Bm], SEQS[A]
                dA_ps = psump.tile([128, 512], dt.float32, tag="bank",
                                   name="dApsum")[:16, :TA]
                dB_ps = psump.tile([128, 512], dt.float32, tag="bank",
                                   name="dBpsum")[:16, :TB]
                for kc in range(8):
                    pAA, pAB, pBB, pBA = prods[kc]
                    nc.tensor.matmul(dA_ps, seg_t[:, kc, :], pAA,
                                     start=(kc == 0), stop=False)
                    nc.tensor.matmul(dA_ps, seg_t[:, kc, :], pAB,
                                     start=False, stop=(kc == 7))
                    dBv = dB_ps.rearrange("h (b s) -> h b s", b=BL)[:, :, :SA]
                    if kc == 0:
                        nc.tensor.matmul(dB_ps, seg_t[:, kc, :], pBB,
                                         start=True, stop=False)
                        nc.tensor.matmul(dBv, seg_t[:, kc, :], aview(pBA),
                                         start=False, stop=False, skip_group_check=True)
                    else:
                        nc.tensor.matmul(dBv, seg_t[:, kc, :], aview(pBA),
                                         start=False, stop=False, skip_group_check=True)
                        nc.tensor.matmul(dB_ps, seg_t[:, kc, :], pBB,
                                         start=False, stop=(kc == 7))
                nc.scalar.activation(wA_t[:16, :], dA_ps, AF.Sigmoid,
                                     bias=nb_t[:16, pi:pi + 1], scale=float(cinv))
                nc.scalar.activation(wB_t[:16, :], dB_ps, AF.Sigmoid,
                                     scale=float(cinv))

            def reps_stage(pi, wA_t, wB_t):
                A, Bm = PAIRS[pi]
                TA, TB = TOKS[A], TOKS[Bm]
                reps = []
                for kc in range(8):
                    rA_ps = psump.tile([128, 512], dt.float32, tag="bank",
                                       name="rApsum")[:, :TA]
                    nc.tensor.matmul(rA_ps, segt_t[:, kc, :], wA_t, start=True, stop=True)
                    rA = repp.tile([128, 154], dt.bfloat16, tag=f"ra{kc}", name="ra")
                    nc.vector.tensor_copy(rA, rA_ps)
                    rB_ps = psump.tile([128, 512], dt.float32, tag="bank",
                                       name="rBpsum")[:, :TB]
                    nc.tensor.matmul(rB_ps, segt_t[:, kc, :], wB_t, start=True, stop=True)
                    rB = repp.tile([128, 512], dt.bfloat16, tag=f"rb{kc}", name="rb")
                    nc.vector.tensor_copy(rB, rB_ps)
                    reps.append((rA, rB))
                return reps

            def attn_ctx_b(pi, reps, atp):
                A, Bm = PAIRS[pi]
                SA = SEQS[A]
                for kc in range(8):
                    rA, rB = reps[kc]
                    # ctxB = wB*vB everywhere; += vA - wB*vA on valid cols
                    t2 = atp.tile([128, 154], dt.bfloat16, tag="pa")
                    nc.vector.tensor_mul(aview(t2), bviewv(rB, SA),
                                         aview(qsl(A, 16 + kc)))
                    t3 = atp.tile([128, 154], dt.bfloat16, tag="pa")
                    nc.vector.tensor_sub(t3, qsl(A, 16 + kc), t2)
                    nc.vector.tensor_mul(qsl(Bm, kc), rB, qsl(Bm, 16 + kc))
                    nc.vector.tensor_add(bviewv(qsl(Bm, kc), SA),
                                         bviewv(qsl(Bm, kc), SA), aview(t3))

            def attn_ctx_a(pi, reps, atp):
                A, Bm = PAIRS[pi]
                SA = SEQS[A]
                for kc in range(8):
                    rA, rB = reps[kc]
                    # ctxA = wA*(vA - vB) + vB  (written over the Q chunks)
                    t1 = atp.tile([128, 154], dt.bfloat16, tag="pa")
                    nc.vector.tensor_sub(aview(t1), aview(qsl(A, 16 + kc)),
                                         bviewv(qsl(Bm, 16 + kc), SA))
                    nc.vector.tensor_mul(t1, t1, rA)
                    nc.vector.tensor_add(aview(qsl(A, kc)), aview(t1),
                                         bviewv(qsl(Bm, 16 + kc), SA))

            def wout_stage(m, wo_m, outp):
                T = TOKS[m]
                off = 0 if m != 1 else TOKS[0]
                src = qk[0] if m <= 1 else qk[m]
                for tci in range((T + 127) // 128):
                    t0 = tci * 128
                    tcs = min(128, T - t0)
                    o_ps = [psump.tile([128, 512], dt.float32, tag="bank",
                                       name="opsum")[:tcs, :] for _ in range(2)]
                    for kc in range(8):
                        for nh in range(2):
                            nc.tensor.matmul(o_ps[nh],
                                             src[:, kc, off + t0:off + t0 + tcs],
                                             wo_m[:, kc, nh * 512:(nh + 1) * 512],
                                             start=(kc == 0), stop=(kc == 7))
                    o_sb = outp.tile([128, D], dt.bfloat16, tag="ot", name="osb")[:tcs, :]
                    for nh in range(2):
                        nc.scalar.copy(o_sb[:, nh * 512:(nh + 1) * 512], o_ps[nh])
                    r = 0
                    while r < tcs:   # <=2 contiguous (batch, seq) runs
                        tok = t0 + r
                        b, s = divmod(tok, SEQS[m])
                        run = min(tcs - r, SEQS[m] - s)
                        orow = b * TOTSEQ + OUT_OFF[m] + s
                        nc.sync.dma_start(out=out.ap()[orow:orow + run, :],
                                          in_=o_sb[r:r + run, :])
                        r += run

            with tc.tile_pool(name="attn", bufs=4) as atp, \
                 tc.tile_pool(name="attw", bufs=2) as awp:
                qk[0] = qkvp.tile([128, NQC, TCLIP], dt.bfloat16, tag="qk01",
                                  name="qk01")
                qk[2] = qkvp.tile([128, NQC, TOKS[2]], dt.bfloat16, tag="qk2",
                                  name="qk2")
                qk[3] = qkvp.tile([128, NQC, TOKS[3]], dt.bfloat16, tag="qk3",
                                  name="qk3")
                # pair-0 sigmoid weight tiles, zeroed early
                wA0 = awp.tile([128, 154], dt.bfloat16, tag="wa", name="wa")
                nc.vector.memset(wA0[:], 0.0)
                wB0 = awp.tile([128, 512], dt.bfloat16, tag="wb", name="wb")
                nc.vector.memset(wB0[:], 0.0)

                qkv_t5(2, range(NQC))
                prods0, prods1 = {}, {}
                qkv_clip(prods_pi=0, prods=prods0)
                score_sig(0, prods0, wA0, wB0)          # sigmoid0 on ACT
                qkv_t5(3, range(0, 8))
                reps0 = reps_stage(0, wA0, wB0)
                wA1 = awp.tile([128, 154], dt.bfloat16, tag="wa", name="wa")
                nc.vector.memset(wA1[:], 0.0)
                wB1 = awp.tile([128, 512], dt.bfloat16, tag="wb", name="wb")
                nc.vector.memset(wB1[:], 0.0)
                qkv_t5(3, range(8, NQC), prods_pi=1, prods=prods1)
                attn_ctx_b(0, reps0, atp)               # DVE under t5(3) tail
                attn_ctx_a(0, reps0, atp)
                wqkvp_cm.__exit__(None, None, None)
                pzp_cm.__exit__(None, None, None)
                with tc.tile_pool(name="woutp", bufs=1, side="right") as wop, \
                     tc.tile_pool(name="outp", bufs=3, side="right") as outp:
                    wo = {2: wo2_t}
                    for m in [0, 3, 1]:
                        wo[m] = wop.tile([128, 8, D], dt.bfloat16,
                                         tag=f"wo{m}", name=f"wo{m}")
                        nc.sync.dma_start(wo[m][:], wout.ap()[m * D:(m + 1) * D, :]
                                          .rearrange("(k p) n -> p k n", p=128))
                    score_sig(1, prods1, wA1, wB1)      # sigmoid1 under wout(2)
                    wout_stage(2, wo[2], outp)
                    reps1 = reps_stage(1, wA1, wB1)
                    attn_ctx_b(1, reps1, atp)           # DVE under wout(0)
                    attn_ctx_a(1, reps1, atp)
                    wout_stage(0, wo[0], outp)
                    wout_stage(3, wo[3], outp)
                    wout_stage(1, wo[1], outp)
            repp_cm.__exit__(None, None, None)
            prodp_cm.__exit__(None, None, None)
            wo2p_cm.__exit__(None, None, None)

    nc.compile()
    return nc


def _prep(inputs):
    """Host-side preprocessing: bf16/fp8 casts, bias folding, layout prep."""
    f32 = np.float32
    names = ["clip_l", "clip_g", "t5_l", "t5_g"]
    W = {k: np.asarray(v) for k, v in inputs.items()}

    temp = float(np.abs(W["temperature"]))
    cinv = 1.0 / (np.sqrt(HD) * temp)
    betas = np.asarray(W["betas"], f32)
    nbeta = [-float(betas[0]), -float(betas[1])]
    a_gate = [float(1.0 / (1.0 + np.exp(-W["alphas"][m]))) for m in range(M)]

    wqkv = np.concatenate([W["Wq"], W["Wk"], W["Wv"]], axis=1).astype(f32)
    emb = W["emb"].astype(f32)
    bqkv_full = emb @ wqkv + np.concatenate([W["bq"], W["bk"], W["bv"]])[None, :]

    # gate weights: fp8, host-scaled by S_G, DoubleRow-packed
    wg1 = W["Wg1"].astype(f32) * S_G           # [M, 1024, 256]
    # wg18[p, (m j h two c)] = wg1[m, (2j+two)*128+p, h*128+c]
    wg18 = wg1.reshape(M, 4, 2, 128, 2, 128)   # [m, j, two, p, h, c]
    wg18 = wg18.transpose(3, 0, 1, 4, 2, 5).reshape(128, M * 2048)
    wg2 = W["Wg2"].astype(f32) * S_G           # [M, 256, 1]
    # wg28[p, (m two c)] = wg2[m, two*128+p, 0]  (replicated over c)
    wg28 = np.repeat(wg2.reshape(M, 2, 128, 1).transpose(2, 0, 1, 3),
                     128, axis=3).reshape(128, M * 256)

    shared = {
        "wg18": wg18.astype(F8),
        "wg28": wg28.astype(F8),
        "wqkv": wqkv.astype(BF16),
        "wout": W["Wout"].reshape(M * D, D).astype(BF16),
    }
    for m, nm in enumerate(names):
        shared[f"wp{m}"] = W[f"Wp_{nm}"].astype(BF16)

    # packed per-partition constants: f32 [128,142] and bf16 [128,1152]
    cf = np.zeros((128, 142), f32)
    for m, nm in enumerate(names):
        cf[:, m * 8:(m + 1) * 8] = W[f"bp_{nm}"].astype(f32).reshape(8, 128).T
        cf[:, 32 + m * 2:32 + (m + 1) * 2] = \
            1.702 * W["bg1"][m].astype(f32).reshape(2, 128).T
        cf[:, 40 + m] = float(W["bg2"][m, 0])
        cf[:, 44 + m * NQC:44 + (m + 1) * NQC] = bqkv_full[m].astype(f32)\
            .reshape(NQC, 128).T
    cf[:, 140] = nbeta[0]
    cf[:, 141] = nbeta[1]
    cb = np.zeros((128, 1152), f32)
    for kc in range(8):
        for j in range(128):
            h = 2 * kc + j // 64
            cb[j, kc * 16 + h] = 1.0           # seg
            cb[h, 128 + kc * 128 + j] = 1.0    # segt
    shared["constf"] = cf
    shared["constb"] = cb.astype(BF16)

    in_maps = []
    for c in range(NCORES):
        im = dict(shared)
        for m, nm in enumerate(names):
            xs = np.asarray(W[f"x_{nm}"])[c * BL:(c + 1) * BL].reshape(TOKS[m], DIMS[m])
            im[f"x{m}"] = np.ascontiguousarray(xs.T).astype(BF16)
        in_maps.append(im)
    return in_maps, cinv, nbeta, a_gate


def kernel(**inputs):
    import sys
    if '/opt/trn_rl_repo' not in sys.path:
        sys.path.insert(0, '/opt/trn_rl_repo')
    from concourse.bass_utils import run_bass_kernel_spmd

    in_maps, cinv, nbeta, a_gate = _prep(inputs)
    key = (round(cinv, 9), round(nbeta[0], 9), round(nbeta[1], 9),
           tuple(round(a, 9) for a in a_gate))
    if key not in _cache:
        _cache[key] = _build(cinv, nbeta, a_gate)
    nc = _cache[key]

    res = run_bass_kernel_spmd(nc, in_maps, list(range(NCORES)))
    outs = [np.asarray(res.results[c]["out"], dtype=np.float32).reshape(BL, TOTSEQ, D)
            for c in range(NCORES)]
    full = np.concatenate(outs, axis=0)
    # bout is additive at the very end; apply on host (exact)
    bout = np.asarray(inputs["bout"], np.float32)
    for m in range(M):
        sl = slice(OUT_OFF[m], OUT_OFF[m] + SEQS[m])
        full[:, sl, :] += bout[m][None, None, :]
    return full
